# revision 41
# baseline (speedup 1.0000x reference)
"""Trainium2 Bass kernel for nn_Attention (pooling attention head).

Reference computation (per batch b):
    score[t]  = hidden[t,:] @ W_score @ hidden[-1,:]        # via u = W_score @ h_t
    attn      = softmax(score)
    context   = sum_t attn[t] * hidden[t,:]
    out       = tanh(concat(context, h_t) @ W_out)

Algorithm: reassociate to u = W_score @ h_t (tiny), then one streaming
pass over hidden_states: score = hid @ u (free-axis mul+reduce),
softmax, context accumulated with P-stationary matmuls.

Sharding: data-parallel over batch, 8 batches per NeuronCore, no
collectives. Each core returns its [8, 128] slice of the output.

Layout: partition p holds t-rows p*16 .. p*16+15; column j of S/P maps
to t = p*16 + j. Softmax is order-agnostic and the context contraction
sums over all (p, j), so the remapping is transparent.

Score: 4 wide DVE muls ([128,4,512], fp16 products) per batch; t-tiles
0-7 reduced by a wide DVE add-tree + segmented reduce, t-tiles 8-15 by
8 ACT copy-accums. u is broadcast per batch into a rotating pool tile
via a DRAM-sourced replicating DMA (a single shared tile read by
several engines at once caused SBUF port contention in an earlier
version).

Engine budget per batch (~9-11us DMA of fp32 hidden, "ridge"):
  SWDGE:  2x 2MB cast-DMA (fp32->bf16 inline)
  DVE:    4x wide mul + add-tree + softmax stats   ~9.0us
  ACT:    8x copy-accum + exp + ctx norm           ~7.7us
  PE:     16x N=512 P-stationary context matmuls   ~4.5us
  Sync:   u broadcast + ctx-row moves (HWDGE)
"""

import os

os.environ.setdefault("MYCRO_LOCAL_CACHE", "1")

from contextlib import ExitStack

import numpy as np

import concourse.bass as bass
import concourse.bass_isa as bass_isa
import concourse.tile as tile
from concourse import bacc, mybir
from concourse.bass_utils import run_bass_kernel_spmd
from concourse.masks import make_identity

B, T, H, UNITS = 64, 2048, 512, 128
NCORES = 8
BL = B // NCORES  # local batches per core
NT = T // 128  # 16 t-tiles per batch

F32 = mybir.dt.float32
BF16 = mybir.dt.bfloat16
F16 = mybir.dt.float16


def _kernel_body(tc: tile.TileContext, out, hs, ws, wo):
    nc = tc.nc
    AF = mybir.ActivationFunctionType
    with ExitStack() as ctx:
        singles = ctx.enter_context(tc.tile_pool(name="singles", bufs=1))
        hid_pool = ctx.enter_context(tc.tile_pool(name="hid", bufs=8))
        work = ctx.enter_context(tc.tile_pool(name="work", bufs=4))
        small = ctx.enter_context(tc.tile_pool(name="small", bufs=2))
        ps_setup = ctx.enter_context(
            tc.tile_pool(name="ps_setup", bufs=2, space="PSUM")
        )
        ps_ctx = ctx.enter_context(tc.tile_pool(name="ps_ctx", bufs=2, space="PSUM"))
        ps_stat = ctx.enter_context(tc.tile_pool(name="ps_stat", bufs=2, space="PSUM"))
        dram = ctx.enter_context(tc.tile_pool(name="dram", bufs=1, space="DRAM"))

        u_dram = dram.tile([BL, H], BF16)
        ident = singles.tile([128, 128], F32)
        make_identity(nc, ident)

        # ---- load weights / last-timestep rows --------------------------
        ws_sb = singles.tile([128, 4, H], F32)  # W_score rows r*128+p
        for r in range(4):
            nc.sync.dma_start(
                out=ws_sb[:, r, :],
                in_=ws.rearrange("(r p) k -> p r k", p=128)[:, r, :],
            )
        wout_sb = singles.tile([128, 8, UNITS], F32)  # W_out rows c*128+p
        nc.sync.dma_start(out=wout_sb, in_=wo.rearrange("(c p) j -> p c j", p=128))
        ht_sb = singles.tile([BL, H], F32)  # h_t = hidden[:, -1, :]
        nc.sync.dma_start(out=ht_sb, in_=hs[:, T - 1, :])

        # ---- W_score^T (PE transposes): wsT_sb[p, kc, m] = W_score[m, kc*128+p]
        wsT_sb = singles.tile([128, 4, H], F32)
        for r in range(4):
            for c in range(4):
                pst = ps_setup.tile([128, 128], F32, tag="setup")
                nc.tensor.transpose(pst, ws_sb[:, r, c * 128 : (c + 1) * 128], ident)
                nc.scalar.copy(wsT_sb[:, c, r * 128 : (r + 1) * 128], pst)

        # ---- h_t^T: htT_sb[p, c, b] = h_t[b, c*128+p]
        htT_sb = singles.tile([128, 4, BL], F32)
        for c in range(4):
            pst = ps_setup.tile([128, BL], F32, tag="setup")
            nc.tensor.transpose(
                pst, ht_sb[:, c * 128 : (c + 1) * 128], ident[:BL, :BL]
            )
            nc.scalar.copy(htT_sb[:, c, :], pst)

        # ---- u rows: u_sb8[b, h] = (W_score @ h_t[b])[h]; stage to DRAM
        psu = ps_setup.tile([BL, H], F32, tag="setup")
        for kc in range(4):
            nc.tensor.matmul(
                psu,
                lhsT=htT_sb[:, kc, :],
                rhs=wsT_sb[:, kc, :],
                start=(kc == 0),
                stop=(kc == 3),
            )
        u_sb8 = singles.tile([BL, H], BF16)
        nc.vector.tensor_copy(out=u_sb8, in_=psu)
        nc.sync.dma_start(out=u_dram, in_=u_sb8)

        # u[b] replicated to all partitions x 4 tile positions, all batches
        # staged up front (off the per-batch critical path; DVE-only
        # reader). Batches 0-1 gate the pipeline ramp, so they bypass the
        # DRAM round-trip: selector-matmul broadcast + on-chip copies
        # (~2us) instead of stage-to-DRAM + replicating DMA (~7us).
        u_bc_all = singles.tile([128, BL, 4, H], BF16)
        ones8 = singles.tile([BL, 128], BF16)
        nc.vector.memset(ones8, 1.0)
        sel_sb = singles.tile([BL, 2, 128], BF16)
        for b in range(2):
            nc.vector.tensor_scalar_mul(
                sel_sb[:, b, :], ones8, ident[:BL, b : b + 1]
            )
            psb = ps_setup.tile([128, H], F32, tag="ubc")
            nc.tensor.matmul(
                psb, lhsT=sel_sb[:, b, :], rhs=u_sb8, start=True, stop=True
            )
            for i in range(4):
                if i % 2 == 0:
                    nc.vector.tensor_copy(out=u_bc_all[:, b, i, :], in_=psb)
                else:
                    nc.scalar.copy(u_bc_all[:, b, i, :], psb)
        for b in range(2, BL):
            nc.sync.dma_start(
                out=u_bc_all[:, b, :, :],
                in_=u_dram[b : b + 1, :].unsqueeze(1).to_broadcast([128, 4, H]),
            )
        # ones row for PE-based partition broadcast of the softmax max
        ones_sb = singles.tile([1, 128], F32)
        nc.vector.memset(ones_sb, 1.0)
        ones16 = singles.tile([1, 128], F16)
        nc.vector.memset(ones16, 1.0)

        # context rows (normalized) collected across batches
        ctx_all = singles.tile([BL, H], F32)

        # ---- main streaming loop, software-pipelined --------------------
        # phase A(b): loads + muls + score reductions -> S(b)
        # phase B(b): softmax stats + context matmuls + norm -> ctx row b
        # emission order A(0), A(1), B(0), A(2), B(1), ... keeps each
        # in-order engine queue free of cross-batch stalls (B(b) waits on
        # cross-engine results; A(b+1) must be able to run first).
        state = {}

        def warm_pe(dep_tile):
            # tiny matmul reading the freshest tile: keeps the PE HAM
            # window busy between context bursts without real work
            wps = ps_stat.tile([128, 8], F32, tag="stat")
            nc.tensor.matmul(
                wps, lhsT=ones16, rhs=dep_tile, start=True, stop=True
            )

        def phase_a(b):
            u_bc4 = u_bc_all[:, b, :, :]
            # whole-batch load with inline fp32->bf16 cast (SWDGE),
            # split in two halves for finer pipelining
            hs_v = hs[b].rearrange("(p n) h -> p n h", p=128)
            hid_halves = []
            prod_halves = []
            for half in range(2):
                hid_bf = hid_pool.tile([128, NT // 2, H], BF16, tag="hid")
                nc.gpsimd.dma_start(
                    out=hid_bf, in_=hs_v[:, half * 8 : half * 8 + 8, :]
                )
                hid_halves.append(hid_bf)
                prod8 = work.tile([128, NT // 2, H], F16, tag=f"prod{half}", bufs=3)
                for q in range(2):
                    nc.vector.tensor_mul(
                        prod8[:, q * 4 : (q + 1) * 4, :],
                        hid_bf[:, q * 4 : (q + 1) * 4, :],
                        u_bc4,
                    )
                prod_halves.append(prod8)

            S = small.tile([128, NT], F32, tag="S", bufs=3)
            # t-tiles 0-7: wide DVE add-tree + segmented reduce
            p0 = prod_halves[0]
            t1 = work.tile([128, 8, H // 2], F16, tag="t1", bufs=2)
            nc.vector.tensor_add(t1, p0[:, :, 0 : H // 2], p0[:, :, H // 2 : H])
            warm_pe(t1[0:1, 0, 0:8])
            t2 = work.tile([128, 8, H // 4], F16, tag="t2", bufs=2)
            nc.vector.tensor_add(t2, t1[:, :, 0 : H // 4], t1[:, :, H // 4 : H // 2])
            t3 = work.tile([128, 8, H // 8], F16, tag="t3", bufs=2)
            nc.vector.tensor_add(t3, t2[:, :, 0 : H // 8], t2[:, :, H // 8 : H // 4])
            nc.vector.reduce_sum(S[:, 0:8], t3, axis=mybir.AxisListType.X)
            warm_pe(t3[0:1, 0, 0:8])
            state[b] = (hid_halves, prod_halves, S)

        def phase_a2(b):
            # t-tiles 8-15: ACT copy-accum per column. Emitted AFTER the
            # previous batch's phase B so its exp/norm (critical path) are
            # not queued behind these slack-tolerant accumulations on ACT.
            _, prod_halves, S = state[b]
            dump = work.tile([128, H], F16, tag="dump_a", bufs=1)
            for i in range(8):
                nc.scalar.activation(
                    dump,
                    prod_halves[1][:, i, :],
                    AF.Copy,
                    accum_out=S[:, 8 + i : 9 + i],
                )

        def phase_b(b):
            hid_halves, _, S = state.pop(b)
            # softmax over all 2048 scores; cross-partition stats via a
            # single GPSIMD all-reduce per statistic (replaces the
            # PE-transpose + reduce + ones-matmul broadcast chains)
            m_row = small.tile([128, 1], F32, tag="m_row")
            nc.vector.reduce_max(m_row, S, axis=mybir.AxisListType.X)
            M_bc = small.tile([128, 1], F32, tag="M_bc")
            nc.gpsimd.partition_all_reduce(
                M_bc, m_row, channels=128, reduce_op=bass_isa.ReduceOp.max
            )
            nm = small.tile([128, 1], F32, tag="nm")
            nc.vector.tensor_scalar_mul(nm, M_bc, -1.0)

            P = small.tile([128, NT], BF16, tag="P", bufs=3)
            nc.scalar.activation(P, S, AF.Exp, bias=nm, scale=1.0)
            warm_pe(P[0:1, 0:8].bitcast(F16))
            l_row = small.tile([128, 1], F32, tag="l_row")
            nc.vector.reduce_sum(l_row, P, axis=mybir.AxisListType.X)
            L_bc = small.tile([128, 1], F32, tag="L_bc")
            nc.gpsimd.partition_all_reduce(
                L_bc, l_row, channels=128, reduce_op=bass_isa.ReduceOp.add
            )
            Linv = small.tile([1, 1], F32, tag="Linv")
            nc.vector.reciprocal(Linv, L_bc[0:1, :])

            # context row via P-stationary matmuls. The [128,1] stationary
            # columns alternate between PE col-groups 0 and 32 (with the
            # matching PSUM output rows) so each LDWEIGHTS overlaps the
            # in-flight matmul -- a single col-group serializes every
            # ldweights+matmul pair at the isolated-latency rate.
            ps2 = ps_ctx.tile([33, H], F32, tag="ctx")
            for j in range(NT):
                g = 32 * (j % 2)
                nc.tensor.matmul(
                    ps2[g : g + 1, :],
                    lhsT=P[:, j : j + 1],
                    rhs=hid_halves[j // 8][:, j % 8, :],
                    start=(j < 2),
                    stop=(j >= NT - 2),
                    tile_position=(0, g),
                )
            # combine the two rows (only one matmul input may live in PSUM:
            # stage row 32 through SBUF on ACT), normalize by 1/L
            sbB_row = small.tile([1, H], F32, tag="sbB")
            nc.scalar.copy(sbB_row, ps2[32:33, :])
            ctxu_row = small.tile([1, H], F32, tag="ctxu")
            nc.vector.tensor_add(ctxu_row, ps2[0:1, :], sbB_row)
            ctxn_row = small.tile([1, H], F32, tag="ctxn")
            nc.scalar.mul(ctxn_row, ctxu_row, Linv)
            nc.sync.dma_start(out=ctx_all[b : b + 1, :], in_=ctxn_row)

        phase_a(0)
        phase_a2(0)
        phase_a(1)
        for b in range(BL):
            if b + 2 < BL:
                phase_a(b + 2)
            phase_b(b)
            if b + 1 < BL:
                phase_a2(b + 1)

        # ---- final: out = tanh(concat(ctx, h_t) @ W_out) ----------------
        ctxT_sb = singles.tile([128, 4, BL], F32)
        for c in range(4):
            pst = ps_setup.tile([128, BL], F32, tag="setup")
            nc.tensor.transpose(
                pst, ctx_all[:, c * 128 : (c + 1) * 128], ident[:BL, :BL]
            )
            nc.scalar.copy(ctxT_sb[:, c, :], pst)
        psum_out = ps_setup.tile([BL, UNITS], F32, tag="setup")
        for c in range(8):
            lhsT = ctxT_sb[:, c, :] if c < 4 else htT_sb[:, c - 4, :]
            nc.tensor.matmul(
                psum_out,
                lhsT=lhsT,
                rhs=wout_sb[:, c, :],
                start=(c == 0),
                stop=(c == 7),
            )
        y_sb = small.tile([BL, UNITS], F32, tag="y")
        nc.scalar.activation(y_sb, psum_out, AF.Tanh)
        nc.sync.dma_start(out=out, in_=y_sb)


def build_nc():
    nc = bacc.Bacc(
        "TRN2",
        target_bir_lowering=False,
        debug=False,
        enable_asserts=False,
        num_devices=NCORES,
    )
    hs = nc.dram_tensor(
        "hidden_states", [BL, T, H], F32, kind="ExternalInput"
    ).ap()
    ws = nc.dram_tensor("W_score", [H, H], F32, kind="ExternalInput").ap()
    wo = nc.dram_tensor("W_out", [2 * H, UNITS], F32, kind="ExternalInput").ap()
    out = nc.dram_tensor("out", [BL, UNITS], F32, kind="ExternalOutput").ap()

    with tile.TileContext(nc) as tc:
        _kernel_body(tc, out, hs, ws, wo)
    nc.compile()
    return nc


_NC = None


def _get_nc():
    global _NC
    if _NC is None:
        _NC = build_nc()
    return _NC


def make_in_maps(hidden_states, W_score, W_out):
    hidden_states = np.ascontiguousarray(
        np.asarray(hidden_states, dtype=np.float32)
    )
    W_score = np.ascontiguousarray(np.asarray(W_score, dtype=np.float32))
    W_out = np.ascontiguousarray(np.asarray(W_out, dtype=np.float32))
    return [
        {
            "hidden_states": hidden_states[i * BL : (i + 1) * BL],
            "W_score": W_score,
            "W_out": W_out,
        }
        for i in range(NCORES)
    ]


def kernel(hidden_states, W_score, W_out):
    nc = _get_nc()
    in_maps = make_in_maps(hidden_states, W_score, W_out)
    res = run_bass_kernel_spmd(nc, in_maps, core_ids=list(range(NCORES)))
    return np.concatenate([res.results[i]["out"] for i in range(NCORES)], axis=0)


# revision 44
# speedup vs baseline: 1.0878x; 1.0878x over previous
"""Trainium2 Bass kernel for nn_Attention (pooling attention head).

Reference computation (per batch b):
    score[t]  = hidden[t,:] @ W_score @ hidden[-1,:]        # via u = W_score @ h_t
    attn      = softmax(score)
    context   = sum_t attn[t] * hidden[t,:]
    out       = tanh(concat(context, h_t) @ W_out)

Algorithm: reassociate to u = W_score @ h_t (tiny), then one streaming
pass over hidden_states: score = hid @ u (free-axis mul+reduce),
softmax, context accumulated with P-stationary matmuls.

Sharding: data-parallel over batch, 8 batches per NeuronCore, no
collectives. Each core returns its [8, 128] slice of the output.

Layout: partition p holds t-rows p*16 .. p*16+15; column j of S/P maps
to t = p*16 + j. Softmax is order-agnostic and the context contraction
sums over all (p, j), so the remapping is transparent.

Score: 4 wide DVE muls ([128,4,512], fp16 products) per batch; t-tiles
0-7 reduced by a wide DVE add-tree + segmented reduce, t-tiles 8-15 by
8 ACT copy-accums. u is broadcast per batch into a rotating pool tile
via a DRAM-sourced replicating DMA (a single shared tile read by
several engines at once caused SBUF port contention in an earlier
version).

Engine budget per batch (~9-11us DMA of fp32 hidden, "ridge"):
  SWDGE:  2x 2MB cast-DMA (fp32->bf16 inline)
  DVE:    4x wide mul + add-tree + softmax stats   ~9.0us
  ACT:    8x copy-accum + exp + ctx norm           ~7.7us
  PE:     16x N=512 P-stationary context matmuls   ~4.5us
  Sync:   u broadcast + ctx-row moves (HWDGE)
"""

import os

os.environ.setdefault("MYCRO_LOCAL_CACHE", "1")

from contextlib import ExitStack

import numpy as np

import concourse.bass as bass
import concourse.bass_isa as bass_isa
import concourse.tile as tile
from concourse import bacc, mybir
from concourse.bass_utils import run_bass_kernel_spmd
from concourse.masks import make_identity

B, T, H, UNITS = 64, 2048, 512, 128
NCORES = 8
BL = B // NCORES  # local batches per core
NT = T // 128  # 16 t-tiles per batch

F32 = mybir.dt.float32
BF16 = mybir.dt.bfloat16
F16 = mybir.dt.float16


def _kernel_body(tc: tile.TileContext, out, hs, ws, wo):
    nc = tc.nc
    AF = mybir.ActivationFunctionType
    with ExitStack() as ctx:
        singles = ctx.enter_context(tc.tile_pool(name="singles", bufs=1))
        hid_pool = ctx.enter_context(tc.tile_pool(name="hid", bufs=8))
        work = ctx.enter_context(tc.tile_pool(name="work", bufs=4))
        small = ctx.enter_context(tc.tile_pool(name="small", bufs=2))
        ps_setup = ctx.enter_context(
            tc.tile_pool(name="ps_setup", bufs=2, space="PSUM")
        )
        ps_ctx = ctx.enter_context(tc.tile_pool(name="ps_ctx", bufs=2, space="PSUM"))
        ps_stat = ctx.enter_context(tc.tile_pool(name="ps_stat", bufs=2, space="PSUM"))
        dram = ctx.enter_context(tc.tile_pool(name="dram", bufs=1, space="DRAM"))

        u_dram = dram.tile([BL, H], BF16)
        ident = singles.tile([128, 128], F32)
        make_identity(nc, ident)
        # dummy exp: pull the ~2.7us exp_and_others ACT table load into the
        # idle ramp (otherwise the first softmax exp pays it on batch 0's
        # critical path); all later Copy/Exp/Tanh share this set
        etab = small.tile([1, 1], F32, tag="etab", bufs=1)
        nc.scalar.activation(etab, ident[0:1, 0:1], mybir.ActivationFunctionType.Exp)

        # ---- load weights / last-timestep rows --------------------------
        ws_sb = singles.tile([128, 4, H], F32)  # W_score rows r*128+p
        for r in range(4):
            nc.sync.dma_start(
                out=ws_sb[:, r, :],
                in_=ws.rearrange("(r p) k -> p r k", p=128)[:, r, :],
            )
        wout_sb = singles.tile([128, 8, UNITS], F32)  # W_out rows c*128+p
        nc.sync.dma_start(out=wout_sb, in_=wo.rearrange("(c p) j -> p c j", p=128))
        ht_sb = singles.tile([BL, H], F32)  # h_t = hidden[:, -1, :]
        nc.sync.dma_start(out=ht_sb, in_=hs[:, T - 1, :])

        # ---- W_score^T (PE transposes): wsT_sb[p, kc, m] = W_score[m, kc*128+p]
        wsT_sb = singles.tile([128, 4, H], F32)
        for r in range(4):
            for c in range(4):
                pst = ps_setup.tile([128, 128], F32, tag="setup")
                nc.tensor.transpose(pst, ws_sb[:, r, c * 128 : (c + 1) * 128], ident)
                nc.scalar.copy(wsT_sb[:, c, r * 128 : (r + 1) * 128], pst)

        # ---- h_t^T: htT_sb[p, c, b] = h_t[b, c*128+p]
        htT_sb = singles.tile([128, 4, BL], F32)
        for c in range(4):
            pst = ps_setup.tile([128, BL], F32, tag="setup")
            nc.tensor.transpose(
                pst, ht_sb[:, c * 128 : (c + 1) * 128], ident[:BL, :BL]
            )
            nc.scalar.copy(htT_sb[:, c, :], pst)

        # ---- u rows: u_sb8[b, h] = (W_score @ h_t[b])[h]; stage to DRAM
        psu = ps_setup.tile([BL, H], F32, tag="setup")
        for kc in range(4):
            nc.tensor.matmul(
                psu,
                lhsT=htT_sb[:, kc, :],
                rhs=wsT_sb[:, kc, :],
                start=(kc == 0),
                stop=(kc == 3),
            )
        u_sb8 = singles.tile([BL, H], BF16)
        nc.vector.tensor_copy(out=u_sb8, in_=psu)
        nc.sync.dma_start(out=u_dram, in_=u_sb8)

        # u[b] replicated to all partitions x 4 tile positions, all batches
        # staged up front (off the per-batch critical path; DVE-only reader)
        u_bc_all = singles.tile([128, BL, 4, H], BF16)
        for b in range(BL):
            nc.sync.dma_start(
                out=u_bc_all[:, b, :, :],
                in_=u_dram[b : b + 1, :].unsqueeze(1).to_broadcast([128, 4, H]),
            )
        # ones row for PE-based partition broadcast of the softmax max
        ones_sb = singles.tile([1, 128], F32)
        nc.vector.memset(ones_sb, 1.0)
        ones16 = singles.tile([1, 128], F16)
        nc.vector.memset(ones16, 1.0)

        # context rows (normalized) collected across batches
        ctx_all = singles.tile([BL, H], F32)

        # ---- main streaming loop, software-pipelined --------------------
        # phase A(b): loads + muls + score reductions -> S(b)
        # phase B(b): softmax stats + context matmuls + norm -> ctx row b
        # emission order A(0), A(1), B(0), A(2), B(1), ... keeps each
        # in-order engine queue free of cross-batch stalls (B(b) waits on
        # cross-engine results; A(b+1) must be able to run first).
        state = {}

        def warm_pe(dep_tile):
            # tiny matmul reading the freshest tile: keeps the PE HAM
            # window busy between context bursts without real work
            wps = ps_stat.tile([128, 8], F32, tag="stat")
            nc.tensor.matmul(
                wps, lhsT=ones16, rhs=dep_tile, start=True, stop=True
            )

        def phase_a(b):
            u_bc4 = u_bc_all[:, b, :, :]
            # whole-batch load with inline fp32->bf16 cast (SWDGE),
            # split in two halves for finer pipelining
            hs_v = hs[b].rearrange("(p n) h -> p n h", p=128)
            hid_halves = []
            prod_halves = []
            for half in range(2):
                hid_bf = hid_pool.tile([128, NT // 2, H], BF16, tag="hid")
                nc.gpsimd.dma_start(
                    out=hid_bf, in_=hs_v[:, half * 8 : half * 8 + 8, :]
                )
                hid_halves.append(hid_bf)
                prod8 = work.tile([128, NT // 2, H], F16, tag=f"prod{half}", bufs=3)
                for q in range(2):
                    nc.vector.tensor_mul(
                        prod8[:, q * 4 : (q + 1) * 4, :],
                        hid_bf[:, q * 4 : (q + 1) * 4, :],
                        u_bc4,
                    )
                prod_halves.append(prod8)

            S = small.tile([128, NT], F32, tag="S", bufs=3)
            # t-tiles 0-7: wide DVE add-tree + segmented reduce
            p0 = prod_halves[0]
            t1 = work.tile([128, 8, H // 2], F16, tag="t1", bufs=2)
            nc.vector.tensor_add(t1, p0[:, :, 0 : H // 2], p0[:, :, H // 2 : H])
            warm_pe(t1[0:1, 0, 0:8])
            t2 = work.tile([128, 8, H // 4], F16, tag="t2", bufs=2)
            nc.vector.tensor_add(t2, t1[:, :, 0 : H // 4], t1[:, :, H // 4 : H // 2])
            t3 = work.tile([128, 8, H // 8], F16, tag="t3", bufs=2)
            nc.vector.tensor_add(t3, t2[:, :, 0 : H // 8], t2[:, :, H // 8 : H // 4])
            nc.vector.reduce_sum(S[:, 0:8], t3, axis=mybir.AxisListType.X)
            warm_pe(t3[0:1, 0, 0:8])
            state[b] = (hid_halves, prod_halves, S)

        def phase_a2(b):
            # t-tiles 8-15: ACT copy-accum per column. Emitted AFTER the
            # previous batch's phase B so its exp/norm (critical path) are
            # not queued behind these slack-tolerant accumulations on ACT.
            _, prod_halves, S = state[b]
            # dump to PSUM: ScalarE sits closer to PSUM than SBUF, so the
            # throwaway full-tile write of each copy-accum is cheaper there
            dump = ps_stat.tile([128, H], F32, tag="dmp", bufs=1)
            for i in range(8):
                nc.scalar.activation(
                    dump,
                    prod_halves[1][:, i, :],
                    AF.Copy,
                    accum_out=S[:, 8 + i : 9 + i],
                )

        def phase_b(b):
            hid_halves, _, S = state.pop(b)
            # softmax over all 2048 scores; cross-partition stats via a
            # single GPSIMD all-reduce per statistic (replaces the
            # PE-transpose + reduce + ones-matmul broadcast chains)
            m_row = small.tile([128, 1], F32, tag="m_row")
            nc.vector.reduce_max(m_row, S, axis=mybir.AxisListType.X)
            M_bc = small.tile([128, 1], F32, tag="M_bc")
            nc.gpsimd.partition_all_reduce(
                M_bc, m_row, channels=128, reduce_op=bass_isa.ReduceOp.max
            )
            nm = small.tile([128, 1], F32, tag="nm")
            nc.vector.tensor_scalar_mul(nm, M_bc, -1.0)

            P = small.tile([128, NT], BF16, tag="P", bufs=3)
            nc.scalar.activation(P, S, AF.Exp, bias=nm, scale=1.0)
            warm_pe(P[0:1, 0:8].bitcast(F16))
            l_row = small.tile([128, 1], F32, tag="l_row")
            nc.vector.reduce_sum(l_row, P, axis=mybir.AxisListType.X)
            L_bc = small.tile([128, 1], F32, tag="L_bc")
            nc.gpsimd.partition_all_reduce(
                L_bc, l_row, channels=128, reduce_op=bass_isa.ReduceOp.add
            )
            Linv = small.tile([1, 1], F32, tag="Linv")
            nc.vector.reciprocal(Linv, L_bc[0:1, :])

            # context row via P-stationary matmuls. The [128,1] stationary
            # columns alternate between PE col-groups 0 and 32 (with the
            # matching PSUM output rows) so each LDWEIGHTS overlaps the
            # in-flight matmul -- a single col-group serializes every
            # ldweights+matmul pair at the isolated-latency rate.
            ps2 = ps_ctx.tile([33, H], F32, tag="ctx")
            for j in range(NT):
                g = 32 * (j % 2)
                nc.tensor.matmul(
                    ps2[g : g + 1, :],
                    lhsT=P[:, j : j + 1],
                    rhs=hid_halves[j // 8][:, j % 8, :],
                    start=(j < 2),
                    stop=(j >= NT - 2),
                    tile_position=(0, g),
                )
            # combine the two rows (only one matmul input may live in PSUM:
            # stage row 32 through SBUF on ACT), normalize by 1/L
            sbB_row = small.tile([1, H], F32, tag="sbB")
            nc.scalar.copy(sbB_row, ps2[32:33, :])
            ctxu_row = small.tile([1, H], F32, tag="ctxu")
            nc.vector.tensor_add(ctxu_row, ps2[0:1, :], sbB_row)
            ctxn_row = small.tile([1, H], F32, tag="ctxn")
            nc.scalar.mul(ctxn_row, ctxu_row, Linv)
            nc.sync.dma_start(out=ctx_all[b : b + 1, :], in_=ctxn_row)

        phase_a(0)
        phase_a2(0)
        phase_a(1)
        for b in range(BL):
            if b + 2 < BL:
                phase_a(b + 2)
            phase_b(b)
            if b + 1 < BL:
                phase_a2(b + 1)

        # ---- final: out = tanh(concat(ctx, h_t) @ W_out) ----------------
        ctxT_sb = singles.tile([128, 4, BL], F32)
        for c in range(4):
            pst = ps_setup.tile([128, BL], F32, tag="setup")
            nc.tensor.transpose(
                pst, ctx_all[:, c * 128 : (c + 1) * 128], ident[:BL, :BL]
            )
            nc.scalar.copy(ctxT_sb[:, c, :], pst)
        psum_out = ps_setup.tile([BL, UNITS], F32, tag="setup")
        for c in range(8):
            lhsT = ctxT_sb[:, c, :] if c < 4 else htT_sb[:, c - 4, :]
            nc.tensor.matmul(
                psum_out,
                lhsT=lhsT,
                rhs=wout_sb[:, c, :],
                start=(c == 0),
                stop=(c == 7),
            )
        y_sb = small.tile([BL, UNITS], F32, tag="y")
        nc.scalar.activation(y_sb, psum_out, AF.Tanh)
        nc.sync.dma_start(out=out, in_=y_sb)


def build_nc():
    nc = bacc.Bacc(
        "TRN2",
        target_bir_lowering=False,
        debug=False,
        enable_asserts=False,
        num_devices=NCORES,
    )
    hs = nc.dram_tensor(
        "hidden_states", [BL, T, H], F32, kind="ExternalInput"
    ).ap()
    ws = nc.dram_tensor("W_score", [H, H], F32, kind="ExternalInput").ap()
    wo = nc.dram_tensor("W_out", [2 * H, UNITS], F32, kind="ExternalInput").ap()
    out = nc.dram_tensor("out", [BL, UNITS], F32, kind="ExternalOutput").ap()

    with tile.TileContext(nc) as tc:
        _kernel_body(tc, out, hs, ws, wo)
    nc.compile()
    return nc


_NC = None


def _get_nc():
    global _NC
    if _NC is None:
        _NC = build_nc()
    return _NC


def make_in_maps(hidden_states, W_score, W_out):
    hidden_states = np.ascontiguousarray(
        np.asarray(hidden_states, dtype=np.float32)
    )
    W_score = np.ascontiguousarray(np.asarray(W_score, dtype=np.float32))
    W_out = np.ascontiguousarray(np.asarray(W_out, dtype=np.float32))
    return [
        {
            "hidden_states": hidden_states[i * BL : (i + 1) * BL],
            "W_score": W_score,
            "W_out": W_out,
        }
        for i in range(NCORES)
    ]


def kernel(hidden_states, W_score, W_out):
    nc = _get_nc()
    in_maps = make_in_maps(hidden_states, W_score, W_out)
    res = run_bass_kernel_spmd(nc, in_maps, core_ids=list(range(NCORES)))
    return np.concatenate([res.results[i]["out"] for i in range(NCORES)], axis=0)
